# revision 37
# baseline (speedup 1.0000x reference)
"""Cross-attention Trainium2 kernel (Bass/Tile), data-parallel over batch on 8 cores.

Reference computation per batch element b (no 1/sqrt(d) scaling):
    Q = S2[b] @ Wq            [N2, E]
    K = S1[b] @ Wk            [N1, E]
    V = S1[b] @ Wv            [N1, E]
    A = softmax(Q @ K^T, -1)  [N2, N1]
    out[b] = (A @ V) @ Wo + bo  [N2, D]

Algebraic restructure (exact in real arithmetic):
    Q K^T = S2 (Wq Wk^T) S1^T          -> Wqk = Wq @ Wk^T  [D, D]  (host)
    (A V) Wo = A (S1 (Wv Wo))          -> Wvo = Wv @ Wo    [D, D]  (host)
    rows of A sum to 1, so the bias folds into the value path:
    out = A (S1 Wvo + bo) = E (S1 Wvo + bo) / rowsum(E),  E = exp(scores)
The inner dim (1024) disappears from the device computation entirely:
10.7 GFLOP/core instead of 25.8.

Device layout is fully transposed (feature dims on SBUF partitions):
    host supplies S1T = S1[b].T, S2T = S2[b].T  [D, N]
    phase A: VWo[m, d] = S1 @ Wvo + bo  -> SBUF-resident bf16 [16 mt][128, 512]
      (chunk-0's Q' projection is emitted between VWo groups to fill the
      s1 second-half DMA window; ~8 warmup matmuls + a dummy activation at
      the top absorb the HAM clock-gate ramp and the ACT table load)
    phase B per 512-query chunk:
      Q'T = Wqk^T @ S2T chunk            [d', n]  (16 MMs)
      scoresT tiles  = S1T^T @ Q'T       [m, n]   (64 MMs) -> exp (bf16)
      running esum (DVE adds), UT' = VWo^T-slices @ E accumulated in 4 PSUM
      banks over all 16 m-tiles (64 MMs), ones-matmul partition-reduce of
      esum -> sums row + unnormalized U (bf16) to DRAM; the host divides
      U by the sums (removes the reciprocal/broadcast/multiply tail).
UT' matmuls are emitted with a 2-group lag behind the scores matmuls so the
scalar-engine exp latency is hidden by the in-order PE queue. Evictions and
store triggers are split across the scalar/vector/sync queues.

All matmul operands are float32r (TF32-like 12-bit-mantissa rounding in the
PE, full throughput at moving dim >= 256) except E/VWo which are bf16
(bf16 stationary also gets fast-weight-load: those matmuls hit the ideal
216ns for 512 moving columns vs ~227ns with f32r weights).
"""
import sys

sys.path.insert(0, "/opt/trn_rl_repo")

import numpy as np
from contextlib import ExitStack

P = 128
N_CORES = 8
B = 8          # batch (one element per core)
NQ = 2048      # queries (N2)
NK = 2048      # keys (N1)
D = 512        # query/cross dim
CHUNK = 512    # query-chunk width (moving free dim)
LAG = 2        # UT' emission lag (in m-tile groups) to hide exp latency

_cache = {}


def _build(nq=NQ, nk=NK):
    import concourse.tile as tile
    from concourse import bacc, mybir

    F32 = mybir.dt.float32
    F32R = mybir.dt.float32r
    BF16 = mybir.dt.bfloat16
    Exp = mybir.ActivationFunctionType.Exp
    Copy = mybir.ActivationFunctionType.Copy
    Recip = mybir.ActivationFunctionType.Reciprocal

    n_chunks = nq // CHUNK
    m_tiles = nk // P        # 16 key tiles of 128
    d_tiles = D // P         # 4

    nc = bacc.Bacc("TRN2", target_bir_lowering=False, debug=False)

    S1T = nc.dram_tensor("S1T", [D, nk], F32R, kind="ExternalInput").ap()
    S2T = nc.dram_tensor("S2T", [D, nq], F32R, kind="ExternalInput").ap()
    WQK = nc.dram_tensor("WQK", [D, D], F32R, kind="ExternalInput").ap()
    WVO = nc.dram_tensor("WVO", [D, D], F32R, kind="ExternalInput").ap()
    BOR = nc.dram_tensor("BOR", [1, D], F32, kind="ExternalInput").ap()
    OUT = nc.dram_tensor("OUT", [D, nq], BF16, kind="ExternalOutput").ap()
    SUMS = nc.dram_tensor("SUMS", [nq // CHUNK, CHUNK], F32,
                          kind="ExternalOutput").ap()

    with tile.TileContext(nc) as tc, ExitStack() as ctx, \
            nc.allow_low_precision(reason="float32r/bf16 staging for matmul operands"):
        const = ctx.enter_context(tc.tile_pool(name="const", bufs=1))
        w_pool = ctx.enter_context(tc.tile_pool(name="w_pool", bufs=1))

        # constants
        ones_f = const.tile([P, 1], F32, name="ones_f")
        nc.any.memset(ones_f[:], 1.0)
        ones_r = const.tile([P, 1], F32R, name="ones_r")
        nc.vector.tensor_copy(ones_r[:], ones_f[:])
        bo_sb = const.tile([1, D], F32, name="bo_sb")
        bo_bc = const.tile([P, D], F32, name="bo_bc")

        # PE warmup: ~8 dummy matmuls on memset data so the HAM clock-gate
        # un-throttles during the initial DMA wait instead of during real work
        warm_s = const.tile([P, P], F32, name="warm_s")
        nc.vector.memset(warm_s[:], 0.0)
        warm_m = const.tile([P, CHUNK], F32, name="warm_m")
        nc.vector.memset(warm_m[:], 0.0)
        # dummy activation: pulls the 1.3us ACT_TABLE_LOAD into the startup
        # DMA window instead of blocking chunk0's first qt eviction
        warm_a = const.tile([P, P], F32, name="warm_a")
        nc.scalar.activation(warm_a[:], warm_s[:], Copy)
        NWARM = 7

        # persistent SBUF tensors
        s1t = w_pool.tile([P, d_tiles, nk], F32R, name="s1t")      # 32KB/part
        wqk_t = w_pool.tile([P, d_tiles, D], F32R, name="wqk_t")   # 8KB
        wvo_t = w_pool.tile([P, d_tiles, D], F32R, name="wvo_t")   # 8KB
        vwo = w_pool.tile([P, m_tiles, D], BF16, name="vwo")       # 16KB

        s1_r = S1T.rearrange("(t p) m -> p t m", p=P)
        wqk_r = WQK.rearrange("(t p) d -> p t d", p=P)
        wvo_r = WVO.rearrange("(t p) d -> p t d", p=P)

        # s2 prefetch for all chunks
        s2_pool = ctx.enter_context(tc.tile_pool(name="s2_pool", bufs=n_chunks))
        s2_tiles = []
        qt_pool = ctx.enter_context(tc.tile_pool(name="qt_pool", bufs=2))

        # ps_mm/ps_sum stay open across phase A and the chunks so chunk0's Q'
        # accumulators never hit the pool-close barrier of the phase-A pool.
        # PSUM budget: phase A = ps_mm(3)+ps_sum(1)+ps_vwo(4) = 8 banks;
        # chunks = ps_mm(3)+ps_sum(1)+ps_ut(4) = 8 banks (ps_ut reuses the
        # closed ps_vwo space, first touched well after the barrier clears).
        ps_mm = ctx.enter_context(tc.tile_pool(name="ps_mm", bufs=3, space="PSUM"))
        ps_sum = ctx.enter_context(tc.tile_pool(name="ps_sum", bufs=1, space="PSUM"))

        def emit_qprime(c):
            # Q'T chunk [d'_tile, 128, CHUNK]; eviction on the scalar engine
            # keeps the vector queue free for the previous chunk's tail
            qt_t = qt_pool.tile([P, d_tiles, CHUNK], F32R, name="qt_t", tag="qt")
            for dpt in range(d_tiles):
                accq = ps_mm.tile([P, CHUNK], F32, name="accQ", tag="mm")
                for dt in range(d_tiles):
                    nc.tensor.matmul(
                        accq[:],
                        wqk_t[:, dt, dpt * P:(dpt + 1) * P],
                        s2_tiles[c][:, dt, :],
                        start=(dt == 0), stop=(dt == d_tiles - 1),
                    )
                nc.scalar.activation(qt_t[:, dpt, :], accq[:], Copy)
            return qt_t

        # ---------------- Phase A: VWo = S1 @ Wvo + bo ----------------
        with tc.tile_pool(name="ps_vwo", bufs=4, space="PSUM") as ps_vwo, \
                nc.named_scope("phaseA"):
            # warmup matmuls (see above); result is never read. Shares the
            # vwo pool rotation: finishes long before its bank is reused.
            warm_ps = ps_vwo.tile([P, CHUNK], F32, name="warm_ps", tag="vwo")
            for i in range(NWARM):
                nc.tensor.matmul(warm_ps[:], warm_s[:], warm_m[:],
                                 start=(i == 0), stop=(i == NWARM - 1))
            # DMA order: bias, then interleave wvo/s1t per d-tile (first half
            # of m) so the first accumulation group's operands arrive first,
            # then wqk + chunk-0 s2 (needed right after phase A), then the
            # rest of s1t and the remaining s2 chunks
            # Input loads alternate between the sync and scalar hardware DMA
            # trigger queues so two rings stream in parallel (phase A is
            # input-bound). Odd d-tiles ride the scalar queue.
            nc.sync.dma_start(bo_sb[:], BOR[:, :])
            nc.gpsimd.partition_broadcast(bo_bc[:], bo_sb[:])
            hm = nk // 2
            for dt in range(d_tiles):
                q = nc.sync if dt % 2 == 0 else nc.scalar
                q.dma_start(wvo_t[:, dt, :], wvo_r[:, dt, :])
                q.dma_start(s1t[:, dt, 0:hm], s1_r[:, dt, 0:hm])
            for c in range(n_chunks):
                s2_tiles.append(
                    s2_pool.tile([P, d_tiles, CHUNK], F32R, name="s2_t",
                                 tag="s2"))
            # wqk + s2 chunk0 arrive between the s1 halves: chunk0's Q' is
            # emitted between VWo groups and fills the s1h2 DMA window
            nc.scalar.dma_start(wqk_t[:], wqk_r)
            nc.sync.dma_start(
                s2_tiles[0][:], S2T[:, 0:CHUNK].rearrange("(t p) n -> p t n", p=P))
            for dt in range(d_tiles):
                q = nc.sync if dt % 2 == 0 else nc.scalar
                q.dma_start(s1t[:, dt, hm:nk], s1_r[:, dt, hm:nk])
            for c in range(1, n_chunks):
                q = nc.sync if c % 2 == 0 else nc.scalar
                q.dma_start(
                    s2_tiles[c][:],
                    S2T[:, c * CHUNK:(c + 1) * CHUNK].rearrange(
                        "(t p) n -> p t n", p=P))

            qt_c0 = None
            for g in range(4):
                if g == 2:
                    qt_c0 = emit_qprime(0)
                mts = list(range(g * 4, g * 4 + 4))
                accs = [
                    ps_vwo.tile([P, D], F32, name="acc_vwo", tag="vwo")
                    for _ in mts
                ]
                for dt in range(d_tiles):
                    for j, mt in enumerate(mts):
                        nc.tensor.matmul(
                            accs[j][:], s1t[:, dt, mt * P:(mt + 1) * P],
                            wvo_t[:, dt, :],
                            start=(dt == 0), stop=(dt == d_tiles - 1),
                        )
                for j, mt in enumerate(mts):
                    nc.vector.tensor_add(vwo[:, mt, :], accs[j][:], bo_bc[:])

        # ---------------- Phase B: attention ----------------
        e_pool = ctx.enter_context(tc.tile_pool(name="e_pool", bufs=6))
        out_pool = ctx.enter_context(tc.tile_pool(name="out_pool", bufs=4))
        misc = ctx.enter_context(tc.tile_pool(name="misc", bufs=2))
        ps_ut = ctx.enter_context(tc.tile_pool(name="ps_ut", bufs=4, space="PSUM"))

        for c in range(n_chunks):
          with nc.named_scope(f"chunk{c}"):
            csl = slice(c * CHUNK, (c + 1) * CHUNK)
            qt_t = qt_c0 if c == 0 else emit_qprime(c)

            # scoresT tiles + exp + running esum; UT' lags LAG groups behind
            esum = misc.tile([P, CHUNK], F32R, name="esum", tag="esum")
            ut_list = [
                ps_ut.tile([P, CHUNK], F32, name="ut", tag="ut")
                for _ in range(d_tiles)
            ]
            e_list = []

            def _emit_ut(mt):
                for dt in range(d_tiles):
                    nc.tensor.matmul(
                        ut_list[dt][:],
                        vwo[:, mt, dt * P:(dt + 1) * P],
                        e_list[mt][:],
                        start=(mt == 0), stop=(mt == m_tiles - 1),
                    )

            for mt in range(m_tiles):
                acc_s = ps_mm.tile([P, CHUNK], F32, name="acc_s", tag="mm")
                for dt in range(d_tiles):
                    nc.tensor.matmul(
                        acc_s[:],
                        s1t[:, dt, mt * P:(mt + 1) * P],
                        qt_t[:, dt, :],
                        start=(dt == 0), stop=(dt == d_tiles - 1),
                    )
                e_t = e_pool.tile([P, CHUNK], BF16, name="e_t", tag="e")
                nc.scalar.activation(e_t[:], acc_s[:], Exp)
                e_list.append(e_t)
                if mt == 0:
                    nc.vector.tensor_copy(esum[:], e_t[:])
                else:
                    nc.vector.tensor_add(esum[:], esum[:], e_t[:])
                if mt >= LAG:
                    _emit_ut(mt - LAG)

            # rowsum matmul goes ahead of the trailing UT' groups so the sums
            # row ships while they run. Normalization (U / sums) happens on
            # the host: the device stores unnormalized U in bf16 plus the
            # fp32 sums row, so the chunk tail is just evictions (split
            # across the scalar and vector engines) with no reciprocal chain,
            # and the UT' banks release as early as possible.
            sum_ps = ps_sum.tile([1, CHUNK], F32, name="sum_ps", tag="sum")
            nc.tensor.matmul(sum_ps[:], ones_r[:], esum[:], start=True, stop=True)
            for mt in range(m_tiles - LAG, m_tiles):
                _emit_ut(mt)
            sum_sb = misc.tile([1, CHUNK], F32, name="sum_sb", tag="sumsb")
            nc.vector.tensor_copy(sum_sb[:], sum_ps[:])
            nc.sync.dma_start(SUMS[c:c + 1, :], sum_sb[:])

            # evictions and store-triggers split across the scalar and vector
            # queues so neither the copies nor the DMA dispatches serialize
            for dt in range(d_tiles):
                o_sb = out_pool.tile([P, CHUNK], BF16, name="o_sb", tag="osb")
                if dt % 2 == 0:
                    nc.scalar.activation(o_sb[:], ut_list[dt][:], Copy)
                    nc.scalar.dma_start(OUT[dt * P:(dt + 1) * P, csl], o_sb[:])
                else:
                    nc.vector.tensor_copy(o_sb[:], ut_list[dt][:])
                    nc.sync.dma_start(OUT[dt * P:(dt + 1) * P, csl], o_sb[:])

    nc.compile()
    return nc


def _get_nc(nq=NQ, nk=NK):
    key = (nq, nk)
    if key not in _cache:
        _cache[key] = _build(nq, nk)
    return _cache[key]


def kernel(S1, S2, Wq, Wk, Wv, Wo, bo, _trace=False):
    from concourse.bass_utils import run_bass_kernel_spmd

    S1 = np.asarray(S1, np.float32)
    S2 = np.asarray(S2, np.float32)
    b, nk, _ = S1.shape
    _, nq, _ = S2.shape
    nc = _get_nc(nq, nk)

    wq = np.asarray(Wq, np.float32)
    wk = np.asarray(Wk, np.float32)
    wv = np.asarray(Wv, np.float32)
    wo = np.asarray(Wo, np.float32)
    wqk = np.ascontiguousarray(wq @ wk.T)          # [D, D]
    wvo = np.ascontiguousarray(wv @ wo)            # [D, D]
    bor = np.ascontiguousarray(np.asarray(bo, np.float32).reshape(1, D))

    in_maps = []
    for i in range(b):
        in_maps.append({
            "S1T": np.ascontiguousarray(S1[i].T),
            "S2T": np.ascontiguousarray(S2[i].T),
            "WQK": wqk, "WVO": wvo, "BOR": bor,
        })

    res = run_bass_kernel_spmd(nc, in_maps, list(range(b)), trace=_trace)
    outs = []
    for i in range(b):
        u = np.asarray(res.results[i]["OUT"]).astype(np.float32)   # [D, nq]
        s = np.asarray(res.results[i]["SUMS"]).astype(np.float32)  # [nc, CHUNK]
        u /= s.reshape(1, nq)
        outs.append(u.T)
    out = np.stack(outs)
    if _trace:
        kernel.last_result = res
    return np.ascontiguousarray(out.astype(np.float32))


# revision 38
# speedup vs baseline: 1.0284x; 1.0284x over previous
"""Cross-attention Trainium2 kernel (Bass/Tile), data-parallel over batch on 8 cores.

Reference computation per batch element b (no 1/sqrt(d) scaling):
    Q = S2[b] @ Wq            [N2, E]
    K = S1[b] @ Wk            [N1, E]
    V = S1[b] @ Wv            [N1, E]
    A = softmax(Q @ K^T, -1)  [N2, N1]
    out[b] = (A @ V) @ Wo + bo  [N2, D]

Algebraic restructure (exact in real arithmetic):
    Q K^T = S2 (Wq Wk^T) S1^T          -> Wqk = Wq @ Wk^T  [D, D]  (host)
    (A V) Wo = A (S1 (Wv Wo))          -> Wvo = Wv @ Wo    [D, D]  (host)
    rows of A sum to 1, so the bias folds into the value path:
    out = A (S1 Wvo + bo) = E (S1 Wvo + bo) / rowsum(E),  E = exp(scores)
The inner dim (1024) disappears from the device computation entirely:
10.7 GFLOP/core instead of 25.8.

Device layout is fully transposed (feature dims on SBUF partitions):
    host supplies S1T = S1[b].T, S2T = S2[b].T  [D, N]
    phase A: VWo[m, d] = S1 @ Wvo + bo  -> SBUF-resident bf16 [16 mt][128, 512]
      (chunk-0's Q' projection is emitted between VWo groups to fill the
      s1 second-half DMA window; ~8 warmup matmuls + a dummy activation at
      the top absorb the HAM clock-gate ramp and the ACT table load)
    phase B per 512-query chunk:
      Q'T = Wqk^T @ S2T chunk            [d', n]  (16 MMs)
      scoresT tiles  = S1T^T @ Q'T       [m, n]   (64 MMs) -> exp (bf16)
      running esum (DVE adds), UT' = VWo^T-slices @ E accumulated in 4 PSUM
      banks over all 16 m-tiles (64 MMs), ones-matmul partition-reduce of
      esum -> sums row + unnormalized U (bf16) to DRAM; the host divides
      U by the sums (removes the reciprocal/broadcast/multiply tail).
UT' matmuls are emitted with a 2-group lag behind the scores matmuls so the
scalar-engine exp latency is hidden by the in-order PE queue. Evictions and
store triggers are split across the scalar/vector/sync queues.

All matmul operands are float32r (TF32-like 12-bit-mantissa rounding in the
PE, full throughput at moving dim >= 256) except E/VWo which are bf16
(bf16 stationary also gets fast-weight-load: those matmuls hit the ideal
216ns for 512 moving columns vs ~227ns with f32r weights).
"""
import sys

sys.path.insert(0, "/opt/trn_rl_repo")

import numpy as np
from contextlib import ExitStack

P = 128
N_CORES = 8
B = 8          # batch (one element per core)
NQ = 2048      # queries (N2)
NK = 2048      # keys (N1)
D = 512        # query/cross dim
CHUNK = 512    # query-chunk width (moving free dim)
LAG = 2        # UT' emission lag (in m-tile groups) to hide exp latency

_cache = {}


def _build(nq=NQ, nk=NK):
    import concourse.tile as tile
    from concourse import bacc, mybir

    F32 = mybir.dt.float32
    F32R = mybir.dt.float32r
    BF16 = mybir.dt.bfloat16
    Exp = mybir.ActivationFunctionType.Exp
    Copy = mybir.ActivationFunctionType.Copy
    Recip = mybir.ActivationFunctionType.Reciprocal

    n_chunks = nq // CHUNK
    m_tiles = nk // P        # 16 key tiles of 128
    d_tiles = D // P         # 4

    nc = bacc.Bacc("TRN2", target_bir_lowering=False, debug=False)

    S1T = nc.dram_tensor("S1T", [D, nk], F32R, kind="ExternalInput").ap()
    S2T = nc.dram_tensor("S2T", [D, nq], F32R, kind="ExternalInput").ap()
    WQK = nc.dram_tensor("WQK", [D, D], F32R, kind="ExternalInput").ap()
    WVO = nc.dram_tensor("WVO", [D, D], F32R, kind="ExternalInput").ap()
    BOR = nc.dram_tensor("BOR", [1, D], F32, kind="ExternalInput").ap()
    OUT = nc.dram_tensor("OUT", [D, nq], BF16, kind="ExternalOutput").ap()
    SUMS = nc.dram_tensor("SUMS", [nq // CHUNK, CHUNK], F32,
                          kind="ExternalOutput").ap()

    with tile.TileContext(nc) as tc, ExitStack() as ctx, \
            nc.allow_low_precision(reason="float32r/bf16 staging for matmul operands"):
        const = ctx.enter_context(tc.tile_pool(name="const", bufs=1))
        w_pool = ctx.enter_context(tc.tile_pool(name="w_pool", bufs=1))

        # constants
        ones_f = const.tile([P, 1], F32, name="ones_f")
        nc.any.memset(ones_f[:], 1.0)
        ones_r = const.tile([P, 1], F32R, name="ones_r")
        nc.vector.tensor_copy(ones_r[:], ones_f[:])
        bo_sb = const.tile([1, D], F32, name="bo_sb")
        bo_bc = const.tile([P, D], F32, name="bo_bc")

        # PE warmup: ~8 dummy matmuls on memset data so the HAM clock-gate
        # un-throttles during the initial DMA wait instead of during real work
        warm_s = const.tile([P, P], F32, name="warm_s")
        nc.vector.memset(warm_s[:], 0.0)
        warm_m = const.tile([P, CHUNK], F32, name="warm_m")
        nc.vector.memset(warm_m[:], 0.0)
        # dummy activation: pulls the 1.3us ACT_TABLE_LOAD into the startup
        # DMA window instead of blocking chunk0's first qt eviction
        warm_a = const.tile([P, P], F32, name="warm_a")
        nc.scalar.activation(warm_a[:], warm_s[:], Copy)
        NWARM = 7

        # persistent SBUF tensors
        s1t = w_pool.tile([P, d_tiles, nk], F32R, name="s1t")      # 32KB/part
        wqk_t = w_pool.tile([P, d_tiles, D], F32R, name="wqk_t")   # 8KB
        wvo_t = w_pool.tile([P, d_tiles, D], F32R, name="wvo_t")   # 8KB
        vwo = w_pool.tile([P, m_tiles, D], BF16, name="vwo")       # 16KB

        s1_r = S1T.rearrange("(t p) m -> p t m", p=P)
        wqk_r = WQK.rearrange("(t p) d -> p t d", p=P)
        wvo_r = WVO.rearrange("(t p) d -> p t d", p=P)

        # s2 prefetch for all chunks
        s2_pool = ctx.enter_context(tc.tile_pool(name="s2_pool", bufs=n_chunks))
        s2_tiles = []
        qt_pool = ctx.enter_context(tc.tile_pool(name="qt_pool", bufs=2))

        # ps_mm/ps_sum stay open across phase A and the chunks so chunk0's Q'
        # accumulators never hit the pool-close barrier of the phase-A pool.
        # PSUM budget: phase A = ps_mm(3)+ps_sum(1)+ps_vwo(4) = 8 banks;
        # chunks = ps_mm(3)+ps_sum(1)+ps_ut(4) = 8 banks (ps_ut reuses the
        # closed ps_vwo space, first touched well after the barrier clears).
        ps_mm = ctx.enter_context(tc.tile_pool(name="ps_mm", bufs=3, space="PSUM"))
        ps_sum = ctx.enter_context(tc.tile_pool(name="ps_sum", bufs=1, space="PSUM"))

        def emit_qprime(c):
            # Q'T chunk [d'_tile, 128, CHUNK]; eviction on the scalar engine
            # keeps the vector queue free for the previous chunk's tail
            qt_t = qt_pool.tile([P, d_tiles, CHUNK], F32R, name="qt_t", tag="qt")
            for dpt in range(d_tiles):
                accq = ps_mm.tile([P, CHUNK], F32, name="accQ", tag="mm")
                for dt in range(d_tiles):
                    nc.tensor.matmul(
                        accq[:],
                        wqk_t[:, dt, dpt * P:(dpt + 1) * P],
                        s2_tiles[c][:, dt, :],
                        start=(dt == 0), stop=(dt == d_tiles - 1),
                    )
                nc.scalar.activation(qt_t[:, dpt, :], accq[:], Copy)
            return qt_t

        # ---------------- Phase A: VWo = S1 @ Wvo + bo ----------------
        with tc.tile_pool(name="ps_vwo", bufs=4, space="PSUM") as ps_vwo, \
                nc.named_scope("phaseA"):
            # warmup matmuls (see above); result is never read. Shares the
            # vwo pool rotation: finishes long before its bank is reused.
            warm_ps = ps_vwo.tile([P, CHUNK], F32, name="warm_ps", tag="vwo")
            for i in range(NWARM):
                nc.tensor.matmul(warm_ps[:], warm_s[:], warm_m[:],
                                 start=(i == 0), stop=(i == NWARM - 1))
            # DMA order: bias, then interleave wvo/s1t per d-tile (first half
            # of m) so the first accumulation group's operands arrive first,
            # then wqk + chunk-0 s2 (needed right after phase A), then the
            # rest of s1t and the remaining s2 chunks
            # All input loads ride the sync hardware DMA queue, ordered to
            # match consumption (splitting across trigger queues measured
            # slower: transfers are HBM-limited, not trigger-limited)
            nc.sync.dma_start(bo_sb[:], BOR[:, :])
            nc.gpsimd.partition_broadcast(bo_bc[:], bo_sb[:])
            hm = nk // 2
            for dt in range(d_tiles):
                nc.sync.dma_start(wvo_t[:, dt, :], wvo_r[:, dt, :])
                nc.sync.dma_start(s1t[:, dt, 0:hm], s1_r[:, dt, 0:hm])
            for c in range(n_chunks):
                s2_tiles.append(
                    s2_pool.tile([P, d_tiles, CHUNK], F32R, name="s2_t",
                                 tag="s2"))
            # wqk + s2 chunk0 arrive between the s1 halves: chunk0's Q' is
            # emitted between VWo groups and fills the s1h2 DMA window
            nc.sync.dma_start(wqk_t[:], wqk_r)
            nc.sync.dma_start(
                s2_tiles[0][:], S2T[:, 0:CHUNK].rearrange("(t p) n -> p t n", p=P))
            for dt in range(d_tiles):
                nc.sync.dma_start(s1t[:, dt, hm:nk], s1_r[:, dt, hm:nk])
            for c in range(1, n_chunks):
                nc.sync.dma_start(
                    s2_tiles[c][:],
                    S2T[:, c * CHUNK:(c + 1) * CHUNK].rearrange(
                        "(t p) n -> p t n", p=P))

            qt_c0 = None
            for g in range(4):
                if g == 2:
                    qt_c0 = emit_qprime(0)
                mts = list(range(g * 4, g * 4 + 4))
                accs = [
                    ps_vwo.tile([P, D], F32, name="acc_vwo", tag="vwo")
                    for _ in mts
                ]
                for dt in range(d_tiles):
                    for j, mt in enumerate(mts):
                        nc.tensor.matmul(
                            accs[j][:], s1t[:, dt, mt * P:(mt + 1) * P],
                            wvo_t[:, dt, :],
                            start=(dt == 0), stop=(dt == d_tiles - 1),
                        )
                for j, mt in enumerate(mts):
                    nc.vector.tensor_add(vwo[:, mt, :], accs[j][:], bo_bc[:])

        # ---------------- Phase B: attention ----------------
        e_pool = ctx.enter_context(tc.tile_pool(name="e_pool", bufs=6))
        out_pool = ctx.enter_context(tc.tile_pool(name="out_pool", bufs=4))
        misc = ctx.enter_context(tc.tile_pool(name="misc", bufs=2))
        ps_ut = ctx.enter_context(tc.tile_pool(name="ps_ut", bufs=4, space="PSUM"))

        for c in range(n_chunks):
          with nc.named_scope(f"chunk{c}"):
            csl = slice(c * CHUNK, (c + 1) * CHUNK)
            qt_t = qt_c0 if c == 0 else emit_qprime(c)

            # scoresT tiles + exp + running esum; UT' lags LAG groups behind
            esum = misc.tile([P, CHUNK], F32R, name="esum", tag="esum")
            ut_list = [
                ps_ut.tile([P, CHUNK], F32, name="ut", tag="ut")
                for _ in range(d_tiles)
            ]
            e_list = []

            def _emit_ut(mt):
                for dt in range(d_tiles):
                    nc.tensor.matmul(
                        ut_list[dt][:],
                        vwo[:, mt, dt * P:(dt + 1) * P],
                        e_list[mt][:],
                        start=(mt == 0), stop=(mt == m_tiles - 1),
                    )

            for mt in range(m_tiles):
                acc_s = ps_mm.tile([P, CHUNK], F32, name="acc_s", tag="mm")
                for dt in range(d_tiles):
                    nc.tensor.matmul(
                        acc_s[:],
                        s1t[:, dt, mt * P:(mt + 1) * P],
                        qt_t[:, dt, :],
                        start=(dt == 0), stop=(dt == d_tiles - 1),
                    )
                e_t = e_pool.tile([P, CHUNK], BF16, name="e_t", tag="e")
                nc.scalar.activation(e_t[:], acc_s[:], Exp)
                e_list.append(e_t)
                if mt == 0:
                    nc.vector.tensor_copy(esum[:], e_t[:])
                else:
                    nc.vector.tensor_add(esum[:], esum[:], e_t[:])
                if mt >= LAG:
                    _emit_ut(mt - LAG)

            # rowsum matmul goes ahead of the trailing UT' groups so the sums
            # row ships while they run. Normalization (U / sums) happens on
            # the host: the device stores unnormalized U in bf16 plus the
            # fp32 sums row, so the chunk tail is just evictions (split
            # across the scalar and vector engines) with no reciprocal chain,
            # and the UT' banks release as early as possible.
            sum_ps = ps_sum.tile([1, CHUNK], F32, name="sum_ps", tag="sum")
            nc.tensor.matmul(sum_ps[:], ones_r[:], esum[:], start=True, stop=True)
            for mt in range(m_tiles - LAG, m_tiles):
                _emit_ut(mt)
            sum_sb = misc.tile([1, CHUNK], F32, name="sum_sb", tag="sumsb")
            nc.vector.tensor_copy(sum_sb[:], sum_ps[:])
            nc.sync.dma_start(SUMS[c:c + 1, :], sum_sb[:])

            # evictions and store-triggers split across the scalar and vector
            # queues so neither the copies nor the DMA dispatches serialize
            for dt in range(d_tiles):
                o_sb = out_pool.tile([P, CHUNK], BF16, name="o_sb", tag="osb")
                if dt % 2 == 0:
                    nc.scalar.activation(o_sb[:], ut_list[dt][:], Copy)
                    nc.scalar.dma_start(OUT[dt * P:(dt + 1) * P, csl], o_sb[:])
                else:
                    nc.vector.tensor_copy(o_sb[:], ut_list[dt][:])
                    nc.sync.dma_start(OUT[dt * P:(dt + 1) * P, csl], o_sb[:])

    nc.compile()
    return nc


def _get_nc(nq=NQ, nk=NK):
    key = (nq, nk)
    if key not in _cache:
        _cache[key] = _build(nq, nk)
    return _cache[key]


def kernel(S1, S2, Wq, Wk, Wv, Wo, bo, _trace=False):
    from concourse.bass_utils import run_bass_kernel_spmd

    S1 = np.asarray(S1, np.float32)
    S2 = np.asarray(S2, np.float32)
    b, nk, _ = S1.shape
    _, nq, _ = S2.shape
    nc = _get_nc(nq, nk)

    wq = np.asarray(Wq, np.float32)
    wk = np.asarray(Wk, np.float32)
    wv = np.asarray(Wv, np.float32)
    wo = np.asarray(Wo, np.float32)
    wqk = np.ascontiguousarray(wq @ wk.T)          # [D, D]
    wvo = np.ascontiguousarray(wv @ wo)            # [D, D]
    bor = np.ascontiguousarray(np.asarray(bo, np.float32).reshape(1, D))

    in_maps = []
    for i in range(b):
        in_maps.append({
            "S1T": np.ascontiguousarray(S1[i].T),
            "S2T": np.ascontiguousarray(S2[i].T),
            "WQK": wqk, "WVO": wvo, "BOR": bor,
        })

    res = run_bass_kernel_spmd(nc, in_maps, list(range(b)), trace=_trace)
    outs = []
    for i in range(b):
        u = np.asarray(res.results[i]["OUT"]).astype(np.float32)   # [D, nq]
        s = np.asarray(res.results[i]["SUMS"]).astype(np.float32)  # [nc, CHUNK]
        u /= s.reshape(1, nq)
        outs.append(u.T)
    out = np.stack(outs)
    if _trace:
        kernel.last_result = res
    return np.ascontiguousarray(out.astype(np.float32))
